# revision 1
# baseline (speedup 1.0000x reference)
"""Trainium2 Bass kernel for a recurrent adaptive-LIF SNN.

Network (per reference):
    B=1024, T=100, n_in=120, h1=512, h2=256, n_out=35
    per step t:
        cur1 = x_t @ W1.T + s1 @ Wrec.T
        a1' = rho1*a1 + (1-rho1)*s1
        v1' = alpha1*v1*(1-s1) + (1-alpha1)*cur1
        s1' = (v1' - (1 + beta_a1*a1') > 0)
        cur2 = s1' @ W2.T ; same LIF for layer 2
        vo' = beta_out*vo + (1-beta_out)*(s2' @ W3.T)
    out = mean_t vo(t)

Sharding: data-parallel over batch across 8 cores (128 batch/core),
weights replicated; the sequential T loop is local per core.

Layout: feature-major — [feature -> partitions, batch -> free].  The
recurrent matmuls then consume spike states directly as the moving
operand (no transposes) and per-feature constants (all uniform) become
immediates.

Layer-1 reformulation (exact algebra; P1 := v1' - 1 lives in PSUM):
    u1' = rho1*u1 + s1           (u1 := beta_a1*a1 / cb1, cb1 = beta_a1*(1-rho1))
    s1' = (cb1*u1' < P1)         (== v1' - (1+beta_a1*a1') > 0)
    q1' = (s1'-1)*P1             (== -(v1'-1)*(1-s1'))
    P1  = W1aug@[x;1] + WrecF@s1 + (-a1*I)@q1
      where W1aug has an extra row alpha1-1 against a constant-1 input row,
      WrecF = ((1-alpha1)Wrec).T - alpha1*I, and q1 init = 1 (since v=0).
Layer 2 (unshifted; P2 := v2'):
    t2 = cb2*s2 (ACT);  w2' = rho2*w2 + t2;  s2' = (w2'+1 < P2)
    q2' = (s2'-1)*P2;   P2 = W2s@s1 + (-a2*I)@q2,  q2 init = 0
Output:
    vo_psum = (b*I)@vo + W3s@s2 ; vo = ACT copy;  SUM += I@vo in PSUM
with (1-alpha)/(1-beta_out) folded into W1/Wrec/W2/W3 on the host.
"""

import sys
import numpy as np

sys.path.insert(0, "/opt/trn_rl_repo")

import ml_dtypes

bf16 = ml_dtypes.bfloat16

# Problem constants (hardcoded per contract)
B, T, N_IN, H1, H2, N_OUT = 1024, 100, 120, 512, 256, 35
N_CORES = 8
BC = B // N_CORES  # 128 batch per core
C1 = H1 // 128     # 4 feature chunks, layer 1
C2 = H2 // 128     # 2 feature chunks, layer 2
K1 = N_IN + 1      # x augmented with a constant-one row

_CACHE = {}


def _build(alpha1, rho1, beta_a1, alpha2, rho2, beta_a2, beta_out):
    import concourse.bacc as bacc
    import concourse.mybir as mybir
    import concourse.tile as tile
    from concourse.alu_op_type import AluOpType

    fp32 = mybir.dt.float32
    bft = mybir.dt.bfloat16
    A = AluOpType
    IDENT = mybir.ActivationFunctionType.Identity

    cb1 = float(beta_a1 * (1.0 - rho1))
    cb2 = float(beta_a2 * (1.0 - rho2))

    nc = bacc.Bacc()

    x_d = nc.declare_dram_parameter("x", [K1, T, BC], bft, isOutput=False)
    w1_d = nc.declare_dram_parameter("w1s", [K1, C1, 128], bft, isOutput=False)
    wr_d = nc.declare_dram_parameter("wrecs", [128, C1, C1, 128], bft, isOutput=False)
    w2_d = nc.declare_dram_parameter("w2s", [128, C1, C2, 128], bft, isOutput=False)
    w3_d = nc.declare_dram_parameter("w3s", [128, C2, N_OUT], bft, isOutput=False)
    ai1_d = nc.declare_dram_parameter("negai1", [128, 128], bft, isOutput=False)
    ai2_d = nc.declare_dram_parameter("negai2", [128, 128], bft, isOutput=False)
    i35_d = nc.declare_dram_parameter("i35", [N_OUT, 2, N_OUT], bft, isOutput=False)
    out_d = nc.declare_dram_parameter("out", [N_OUT, BC], fp32, isOutput=True)

    XCH = 10  # x preload chunks
    TP = T // XCH

    with tile.TileContext(nc) as tc:
        with (
            tc.tile_pool(name="wpool", bufs=1) as wpool,
            tc.tile_pool(name="xpool", bufs=1) as xpool,
            tc.tile_pool(name="st1", bufs=3) as st1,
            tc.tile_pool(name="st2", bufs=3) as st2,
            tc.tile_pool(name="tmp", bufs=3) as tmp,
            tc.tile_pool(name="ps1", bufs=3, space="PSUM") as ps1,
            tc.tile_pool(name="ps2", bufs=2, space="PSUM") as ps2,
            tc.tile_pool(name="pso", bufs=2, space="PSUM") as pso,
            tc.tile_pool(name="pssum", bufs=1, space="PSUM") as pssum,
        ):
            # ---- resident weights ----
            w1_s = wpool.tile([K1, C1, 128], bft, tag="w1")
            nc.sync.dma_start(w1_s[:], w1_d[:])
            wr_s = wpool.tile([128, C1, C1, 128], bft, tag="wr")
            nc.sync.dma_start(wr_s[:], wr_d[:])
            w2_s = wpool.tile([128, C1, C2, 128], bft, tag="w2")
            nc.sync.dma_start(w2_s[:], w2_d[:])
            w3_s = wpool.tile([128, C2, N_OUT], bft, tag="w3")
            nc.sync.dma_start(w3_s[:], w3_d[:])
            ai1_s = wpool.tile([128, 128], bft, tag="ai1")
            nc.sync.dma_start(ai1_s[:], ai1_d[:])
            ai2_s = wpool.tile([128, 128], bft, tag="ai2")
            nc.sync.dma_start(ai2_s[:], ai2_d[:])
            # i35[:,0,:] = identity, i35[:,1,:] = beta_out * identity
            i35_s = wpool.tile([N_OUT, 2, N_OUT], bft, tag="i35")
            nc.sync.dma_start(i35_s[:], i35_d[:])

            # ---- x preload in chunks ----
            x_tiles = []
            for i in range(XCH):
                xt = xpool.tile([K1, TP, BC], bft, tag=f"x{i}")
                nc.sync.dma_start(xt[:], x_d[:, i * TP : (i + 1) * TP, :])
                x_tiles.append(xt)

            # ---- initial states ----
            s1 = st1.tile([128, C1 * BC], bft, tag="s1")
            q1 = st1.tile([128, C1 * BC], bft, tag="q1")
            u1 = st1.tile([128, C1 * BC], bft, tag="u1")
            s2 = st2.tile([128, C2 * BC], bft, tag="s2")
            q2 = st2.tile([128, C2 * BC], bft, tag="q2")
            w2st = st2.tile([128, C2 * BC], bft, tag="w2st")
            for z, val in ((s1, 0.0), (q1, 1.0), (u1, 0.0),
                           (s2, 0.0), (q2, 0.0), (w2st, 0.0)):
                nc.vector.memset(z[:], val)
            vo = tmp.tile([N_OUT, BC], bft, tag="vo")
            nc.vector.memset(vo[:], 0.0)

            sum_ps = pssum.tile([N_OUT, BC], fp32, tag="sum")

            for t in range(T):
                xsl = x_tiles[t // TP][:, t % TP, :]

                # ----- P1 = v1' - 1 -----
                p1 = ps1.tile([128, C1 * BC], fp32, tag="p1")
                for m in range(C1):
                    o = p1[:, m * BC : (m + 1) * BC]
                    nc.tensor.matmul(o, w1_s[:, m, :], xsl, start=True, stop=False)
                    for k in range(C1):
                        nc.tensor.matmul(
                            o, wr_s[:, k, m, :], s1[:, k * BC : (k + 1) * BC],
                            start=False, stop=False,
                        )
                    nc.tensor.matmul(
                        o, ai1_s[:], q1[:, m * BC : (m + 1) * BC],
                        start=False, stop=True,
                    )

                # ----- layer 1 state update -----
                u1n = st1.tile([128, C1 * BC], bft, tag="u1")
                nc.vector.scalar_tensor_tensor(
                    u1n[:], u1[:], float(rho1), s1[:], A.mult, A.add
                )
                s1n = st1.tile([128, C1 * BC], bft, tag="s1")
                nc.vector.scalar_tensor_tensor(
                    s1n[:], u1n[:], cb1, p1[:], A.mult, A.is_lt
                )
                q1n = st1.tile([128, C1 * BC], bft, tag="q1")
                nc.vector.scalar_tensor_tensor(
                    q1n[:], s1n[:], 1.0, p1[:], A.subtract, A.mult
                )
                s1, q1, u1 = s1n, q1n, u1n

                # ----- P2 = v2' -----
                p2 = ps2.tile([128, C2 * BC], fp32, tag="p2")
                for m in range(C2):
                    o = p2[:, m * BC : (m + 1) * BC]
                    for k in range(C1):
                        nc.tensor.matmul(
                            o, w2_s[:, k, m, :], s1[:, k * BC : (k + 1) * BC],
                            start=(k == 0), stop=False,
                        )
                    nc.tensor.matmul(
                        o, ai2_s[:], q2[:, m * BC : (m + 1) * BC],
                        start=False, stop=True,
                    )

                # ----- layer 2 state update -----
                p2b = tmp.tile([128, C2 * BC], bft, tag="p2b")
                nc.scalar.activation(p2b[:], p2[:], IDENT)
                t2 = tmp.tile([128, C2 * BC], bft, tag="t2")
                nc.scalar.activation(t2[:], s2[:], IDENT, scale=cb2)
                w2n = st2.tile([128, C2 * BC], bft, tag="w2st")
                nc.vector.scalar_tensor_tensor(
                    w2n[:], w2st[:], float(rho2), t2[:], A.mult, A.add
                )
                s2n = st2.tile([128, C2 * BC], bft, tag="s2")
                nc.vector.scalar_tensor_tensor(
                    s2n[:], w2n[:], 1.0, p2b[:], A.add, A.is_lt
                )
                q2n = st2.tile([128, C2 * BC], bft, tag="q2")
                nc.vector.scalar_tensor_tensor(
                    q2n[:], s2n[:], 1.0, p2b[:], A.subtract, A.mult
                )
                s2, q2, w2st = s2n, q2n, w2n

                # ----- output integrator on PE -----
                yp = pso.tile([N_OUT, BC], fp32, tag="y")
                nc.tensor.matmul(yp[:], i35_s[:, 1, :], vo[:], start=True, stop=False)
                for k in range(C2):
                    nc.tensor.matmul(
                        yp[:], w3_s[:, k, :], s2[:, k * BC : (k + 1) * BC],
                        start=False, stop=(k == C2 - 1),
                    )
                von = tmp.tile([N_OUT, BC], bft, tag="vo")
                nc.scalar.activation(von[:], yp[:], IDENT)
                vo = von

                nc.tensor.matmul(
                    sum_ps[:], i35_s[:, 0, :], vo[:],
                    start=(t == 0), stop=(t == T - 1),
                    skip_group_check=True,
                )

            outf = tmp.tile([N_OUT, BC], fp32, tag="outf")
            nc.vector.tensor_scalar(outf[:], sum_ps[:], 1.0 / T, None, A.mult)
            nc.sync.dma_start(out_d[:], outf[:])

    nc.compile()
    return nc


def _prep_inputs(x, W1, Wrec, W2, W3, alpha1, rho1, beta_a1, alpha2, rho2, beta_a2, beta_out):
    a1 = float(np.asarray(alpha1).reshape(-1)[0])
    a2 = float(np.asarray(alpha2).reshape(-1)[0])
    bo = float(np.asarray(beta_out).reshape(-1)[0])

    w1s = ((1.0 - np.asarray(alpha1, np.float32)[:, None]) * np.asarray(W1, np.float32)).T
    wrs = ((1.0 - np.asarray(alpha1, np.float32)[:, None]) * np.asarray(Wrec, np.float32)).T
    w2s = ((1.0 - np.asarray(alpha2, np.float32)[:, None]) * np.asarray(W2, np.float32)).T
    w3s = ((1.0 - np.asarray(beta_out, np.float32)[:, None]) * np.asarray(W3, np.float32)).T

    # layer-1 shift folds:  WrecF = wrs - a1*I ; W1 gains const row (a1-1)
    wrs = wrs - a1 * np.eye(H1, dtype=np.float32)
    w1aug = np.concatenate(
        [w1s, np.full((1, H1), a1 - 1.0, np.float32)], axis=0
    )  # [121, 512]

    w1_a = np.ascontiguousarray(w1aug.reshape(K1, C1, 128)).astype(bf16)
    wr_a = np.ascontiguousarray(
        wrs.reshape(C1, 128, C1, 128).transpose(1, 0, 2, 3)
    ).astype(bf16)
    w2_a = np.ascontiguousarray(
        w2s.reshape(C1, 128, C2, 128).transpose(1, 0, 2, 3)
    ).astype(bf16)
    w3_a = np.ascontiguousarray(
        w3s.reshape(C2, 128, N_OUT).transpose(1, 0, 2)
    ).astype(bf16)

    nai1 = (-a1 * np.eye(128, dtype=np.float32)).astype(bf16)
    nai2 = (-a2 * np.eye(128, dtype=np.float32)).astype(bf16)
    i35 = np.stack(
        [np.eye(N_OUT, dtype=np.float32), bo * np.eye(N_OUT, dtype=np.float32)], axis=1
    ).astype(bf16)  # [35, 2, 35]

    shared = dict(
        w1s=w1_a, wrecs=wr_a, w2s=w2_a, w3s=w3_a,
        negai1=nai1, negai2=nai2, i35=i35,
    )
    in_maps = []
    for c in range(N_CORES):
        xc = np.asarray(x[c * BC : (c + 1) * BC], np.float32)  # [BC, T, N_IN]
        xfm = xc.transpose(2, 1, 0)  # [N_IN, T, BC]
        xaug = np.concatenate([xfm, np.ones((1, T, BC), np.float32)], axis=0)
        in_maps.append(dict(x=np.ascontiguousarray(xaug).astype(bf16), **shared))
    return in_maps


def kernel(
    x, W1, Wrec, W2, W3,
    alpha1, rho1, beta_a1, alpha2, rho2, beta_a2, beta_out,
    _trace=False,
):
    from concourse.bass_utils import run_bass_kernel_spmd

    key = "nc"
    if key not in _CACHE:
        _CACHE[key] = _build(
            float(np.asarray(alpha1).reshape(-1)[0]),
            float(np.asarray(rho1).reshape(-1)[0]),
            float(np.asarray(beta_a1).reshape(-1)[0]),
            float(np.asarray(alpha2).reshape(-1)[0]),
            float(np.asarray(rho2).reshape(-1)[0]),
            float(np.asarray(beta_a2).reshape(-1)[0]),
            float(np.asarray(beta_out).reshape(-1)[0]),
        )
    nc = _CACHE[key]

    in_maps = _prep_inputs(
        x, W1, Wrec, W2, W3, alpha1, rho1, beta_a1, alpha2, rho2, beta_a2, beta_out
    )
    res = run_bass_kernel_spmd(nc, in_maps, list(range(N_CORES)), trace=_trace)

    out = np.empty((B, N_OUT), np.float32)
    for c in range(N_CORES):
        out[c * BC : (c + 1) * BC] = np.asarray(res.results[c]["out"]).T
    if _trace:
        return out, res
    return out



# revision 6
# speedup vs baseline: 1.3016x; 1.3016x over previous
"""Trainium2 Bass kernel for a recurrent adaptive-LIF SNN.

Network (per reference):
    B=1024, T=100, n_in=120, h1=512, h2=256, n_out=35
    per step t:
        cur1 = x_t @ W1.T + s1 @ Wrec.T
        a1' = rho1*a1 + (1-rho1)*s1 ; v1' = alpha1*v1*(1-s1) + (1-alpha1)*cur1
        s1' = (v1' - (1 + beta_a1*a1') > 0)
        cur2 = s1' @ W2.T ; same LIF for layer 2
        vo' = beta_out*vo + (1-beta_out)*(s2' @ W3.T) ; out = mean_t vo

Sharding: data-parallel over batch across 8 cores (BC=128 per core),
weights replicated; the sequential T loop is local per core.

Reformulation (shifted potential; all engines in their cheap regimes):
    d := LAM*(v - 1 - beta_a*a)  lives in PSUM (LAM=32 keeps fp8
    stationaries in normal range).  Then
        s'      = (d + C > 0)                      spike, 0/1
        gf'     = relu(-(d + C))                   reset carry = -LAM*min(v'-1-..,0)
        d(t+1)  = LAM(1-a)*W@in + LAM*A*s' + (-a)*gf' + C-in-bias
    with A = -(alpha+cb), cb = beta_a*(1-rho), C = LAM*(alpha-1).
    The reset identity v'*(1-s') <-> min(d,0) makes both nonlinear state
    updates UNARY functions of d, so no tensor-tensor products are needed.
    The residual adaptation-trace couplings ((alpha-rho)*cb*u and
    alpha*cb*u*s', coefficients 7.5e-4 / 7.1e-3) are below the fp8
    quantization noise of the weights and are dropped; they are exactly
    zero whenever the layer is not spiking.
    Output integrator is collapsed into per-step weights:
        out = sum_t w_t * W3 @ s2(t),  w_t = (1-beta_out^(T-t))/T,
    accumulated on the PE as one DoubleRow matmul per step.

All matmuls are fp8e4 DoubleRow (two 128-contract planes per
instruction).  Per step: PE ~23 matmuls; DVE computes s1 (from PSUM) and
s2 (from the SBUF reset carry); ACT computes both reset carries.
"""

import sys
import numpy as np

sys.path.insert(0, "/opt/trn_rl_repo")

import ml_dtypes

F8 = ml_dtypes.float8_e4m3

# Problem constants (hardcoded per contract)
B, T, N_IN, H1, H2, N_OUT = 1024, 100, 120, 512, 256, 35
N_CORES = 8
BC = B // N_CORES  # 128 batch per core
C1 = H1 // 128     # 4 feature chunks, layer 1
C2 = H2 // 128     # 2 feature chunks, layer 2
KX = N_IN // 2     # 60: x is stored as two fp8 DoubleRow planes
LAM = 32.0         # PSUM potential scale
S3 = 256.0         # output stationary scale

_CACHE = {}


def _build(a1, r1, b1, a2, r2, b2, bo):
    import concourse.bacc as bacc
    import concourse.mybir as mybir
    import concourse.tile as tile
    from concourse.alu_op_type import AluOpType

    fp32 = mybir.dt.float32
    fp8 = mybir.dt.float8e4
    A = AluOpType
    DR = mybir.MatmulPerfMode.DoubleRow
    RELU = mybir.ActivationFunctionType.Relu

    C1b = LAM * (a1 - 1.0)   # -1.6 : bias folded into the unary state ops
    C2b = LAM * (a2 - 1.0)

    nc = bacc.Bacc()

    x_d = nc.declare_dram_parameter("x", [KX, T, 2, BC], fp8, isOutput=False)
    w1_d = nc.declare_dram_parameter("w1s", [KX, 2, C1, 128], fp8, isOutput=False)
    wr_d = nc.declare_dram_parameter("wrecs", [128, 2, 2, C1, 128], fp8, isOutput=False)
    w2_d = nc.declare_dram_parameter("w2s", [128, 2, 2, C2, 128], fp8, isOutput=False)
    wd1_d = nc.declare_dram_parameter("wdiag1", [128, 2, 128], fp8, isOutput=False)
    wd2_d = nc.declare_dram_parameter("wdiag2", [128, 2, 128], fp8, isOutput=False)
    w3_d = nc.declare_dram_parameter("w3s", [128, T, 2, 64], fp8, isOutput=False)
    out_d = nc.declare_dram_parameter("out", [N_OUT, BC], fp32, isOutput=True)

    XCH = 10  # x preload chunks
    TP = T // XCH

    with tile.TileContext(nc) as tc:
        with (
            tc.tile_pool(name="wpool", bufs=1) as wpool,
            tc.tile_pool(name="xpool", bufs=1) as xpool,
            tc.tile_pool(name="st1", bufs=2) as st1,
            tc.tile_pool(name="st2", bufs=2) as st2,
            tc.tile_pool(name="tmp", bufs=1) as tmp,
            tc.tile_pool(name="ps1", bufs=2, space="PSUM") as ps1,
            tc.tile_pool(name="ps2", bufs=2, space="PSUM") as ps2,
            tc.tile_pool(name="pso", bufs=1, space="PSUM") as pso,
        ):
            # ---- resident weights ----
            w1_s = wpool.tile([KX, 2, C1, 128], fp8, tag="w1")
            nc.sync.dma_start(w1_s[:], w1_d[:])
            wr_s = wpool.tile([128, 2, 2, C1, 128], fp8, tag="wr")
            nc.sync.dma_start(wr_s[:], wr_d[:])
            w2_s = wpool.tile([128, 2, 2, C2, 128], fp8, tag="w2")
            nc.sync.dma_start(w2_s[:], w2_d[:])
            wd1_s = wpool.tile([128, 2, 128], fp8, tag="wd1")
            nc.sync.dma_start(wd1_s[:], wd1_d[:])
            wd2_s = wpool.tile([128, 2, 128], fp8, tag="wd2")
            nc.sync.dma_start(wd2_s[:], wd2_d[:])
            w3_s = wpool.tile([128, T, 2, 64], fp8, tag="w3")
            nc.sync.dma_start(w3_s[:], w3_d[:])
            bias1_s = wpool.tile([128, 1], fp32, tag="bias1")
            nc.vector.memset(bias1_s[:], -C1b)
            bias2_s = wpool.tile([128, 1], fp32, tag="bias2")
            nc.vector.memset(bias2_s[:], -C2b)

            # ---- x preload in chunks ----
            x_tiles = []
            for i in range(XCH):
                xt = xpool.tile([KX, TP, 2, BC], fp8, tag=f"x{i}")
                nc.sync.dma_start(xt[:], x_d[:, i * TP : (i + 1) * TP, :, :])
                x_tiles.append(xt)

            # ---- initial states: sg[:, k, 0, :]=spike, sg[:, k, 1, :]=reset carry
            sg1 = st1.tile([128, C1, 2, BC], fp8, tag="sg1")
            sg2 = st2.tile([128, C2, 2, BC], fp8, tag="sg2")
            nc.vector.memset(sg1[:], 0.0)
            nc.vector.memset(sg2[:], 0.0)
            nc.vector.memset(sg1[:, :, 1, :], LAM)
            nc.vector.memset(sg2[:, :, 1, :], LAM)

            out_ps = pso.tile([64, BC], fp32, tag="out")

            for t in range(T):
                xsl = x_tiles[t // TP][:, t % TP, :, :]

                # ----- d1 accumulation (PSUM, LAM-scaled, pre-bias) -----
                d1 = ps1.tile([128, C1, BC], fp32, tag="d1")
                for m in range(C1):
                    o = d1[:, m, :]
                    nc.tensor.matmul(o, w1_s[:, :, m, :], xsl,
                                     start=True, stop=False, perf_mode=DR)
                    for kp in range(2):
                        nc.tensor.matmul(
                            o, wr_s[:, kp, :, m, :],
                            sg1[:, 2 * kp : 2 * kp + 2, 0, :],
                            start=False, stop=False, perf_mode=DR,
                        )
                    nc.tensor.matmul(o, wd1_s[:], sg1[:, m, :, :],
                                     start=False, stop=True, perf_mode=DR)

                # ----- layer-1 state (unary in d1) -----
                sg1n = st1.tile([128, C1, 2, BC], fp8, tag="sg1")
                nc.vector.tensor_scalar(
                    sg1n[:, :, 0, :], d1[:], C1b, 0.0, A.add, A.is_gt
                )
                nc.scalar.activation(
                    sg1n[:, :, 1, :], d1[:], RELU, bias=bias1_s[:, 0:1], scale=-1.0
                )
                sg1 = sg1n

                # ----- d2 accumulation -----
                d2 = ps2.tile([128, C2, BC], fp32, tag="d2")
                for m in range(C2):
                    o = d2[:, m, :]
                    for kp in range(2):
                        nc.tensor.matmul(
                            o, w2_s[:, kp, :, m, :],
                            sg1[:, 2 * kp : 2 * kp + 2, 0, :],
                            start=(kp == 0), stop=False, perf_mode=DR,
                        )
                    nc.tensor.matmul(o, wd2_s[:], sg2[:, m, :, :],
                                     start=False, stop=True, perf_mode=DR)

                # ----- layer-2 state -----
                sg2n = st2.tile([128, C2, 2, BC], fp8, tag="sg2")
                nc.scalar.activation(
                    sg2n[:, :, 1, :], d2[:], RELU, bias=bias2_s[:, 0:1], scale=-1.0
                )
                nc.vector.tensor_scalar(
                    sg2n[:, :, 0, :], sg2n[:, :, 1, :], 0.0, None, A.is_le
                )
                sg2 = sg2n

                # ----- output accumulation: out += S3*w_t*W3 @ s2(t) -----
                nc.tensor.matmul(
                    out_ps[:], w3_s[:, t, :, :], sg2[:, :, 0, :],
                    start=(t == 0), stop=(t == T - 1), perf_mode=DR,
                    skip_group_check=True,
                )

            outf = tmp.tile([N_OUT, BC], fp32, tag="outf")
            nc.vector.tensor_scalar(outf[:], out_ps[:N_OUT, :], 1.0 / S3, None, A.mult)
            nc.sync.dma_start(out_d[:], outf[:])

    nc.compile()
    return nc


def _prep_inputs(x, W1, Wrec, W2, W3, a1, r1, b1, a2, r2, b2, bo):
    cb1 = b1 * (1.0 - r1)
    cb2 = b2 * (1.0 - r2)
    A1 = -(a1 + cb1)
    A2 = -(a2 + cb2)

    W1f = np.asarray(W1, np.float32) * (LAM * (1.0 - a1))   # [H1, N_IN]
    Wrf = np.asarray(Wrec, np.float32) * (LAM * (1.0 - a1))  # [H1, H1]
    W2f = np.asarray(W2, np.float32) * (LAM * (1.0 - a2))   # [H2, H1]
    W3f = np.asarray(W3, np.float32)                         # [N_OUT, H2]

    # stationary [K, 2, M][k, i, j] = weight(out j, in plane i row k)
    w1s = np.zeros((KX, 2, C1, 128), np.float32)
    for m in range(C1):
        blk = W1f[m * 128 : (m + 1) * 128]          # [128, N_IN]
        w1s[:, 0, m, :] = blk[:, 0:KX].T
        w1s[:, 1, m, :] = blk[:, KX : 2 * KX].T

    wrs = np.zeros((128, 2, 2, C1, 128), np.float32)
    w2s = np.zeros((128, 2, 2, C2, 128), np.float32)
    for m in range(C1):
        blk = Wrf[m * 128 : (m + 1) * 128]          # [128, H1]
        for kp in range(2):
            for i in range(2):
                k = 2 * kp + i
                wrs[:, kp, i, m, :] = blk[:, k * 128 : (k + 1) * 128].T
    for m in range(C2):
        blk = W2f[m * 128 : (m + 1) * 128]          # [128, H1]
        for kp in range(2):
            for i in range(2):
                k = 2 * kp + i
                w2s[:, kp, i, m, :] = blk[:, k * 128 : (k + 1) * 128].T

    eye = np.eye(128, dtype=np.float32)
    wd1 = np.stack([LAM * A1 * eye, -a1 * eye], axis=1)  # [128, 2, 128]
    wd2 = np.stack([LAM * A2 * eye, -a2 * eye], axis=1)

    wt = (1.0 - bo ** (T - np.arange(T, dtype=np.float64))) / T
    w3s = np.zeros((128, T, 2, 64), np.float32)
    for t in range(T):
        sc = np.float32(S3 * wt[t])
        w3s[:, t, 0, :N_OUT] = (sc * W3f[:, 0:128]).T
        w3s[:, t, 1, :N_OUT] = (sc * W3f[:, 128:256]).T

    shared = dict(
        w1s=w1s.astype(F8), wrecs=wrs.astype(F8), w2s=w2s.astype(F8),
        wdiag1=wd1.astype(F8), wdiag2=wd2.astype(F8), w3s=w3s.astype(F8),
    )
    in_maps = []
    for c in range(N_CORES):
        xc = np.asarray(x[c * BC : (c + 1) * BC], np.float32)  # [BC, T, N_IN]
        xfm = xc.transpose(2, 1, 0)                            # [N_IN, T, BC]
        x8 = np.stack([xfm[0:KX], xfm[KX : 2 * KX]], axis=2)   # [KX, T, 2, BC]
        in_maps.append(dict(x=np.ascontiguousarray(x8).astype(F8), **shared))
    return in_maps


def kernel(
    x, W1, Wrec, W2, W3,
    alpha1, rho1, beta_a1, alpha2, rho2, beta_a2, beta_out,
    _trace=False,
):
    from concourse.bass_utils import run_bass_kernel_spmd

    sc = [float(np.asarray(v).reshape(-1)[0]) for v in
          (alpha1, rho1, beta_a1, alpha2, rho2, beta_a2, beta_out)]
    if "nc" not in _CACHE:
        _CACHE["nc"] = _build(*sc)
    nc = _CACHE["nc"]

    in_maps = _prep_inputs(x, W1, Wrec, W2, W3, *sc)
    res = run_bass_kernel_spmd(nc, in_maps, list(range(N_CORES)), trace=_trace)

    out = np.empty((B, N_OUT), np.float32)
    for c in range(N_CORES):
        out[c * BC : (c + 1) * BC] = np.asarray(res.results[c]["out"]).T
    if _trace:
        return out, res
    return out


# revision 7
# speedup vs baseline: 1.3238x; 1.0171x over previous
"""Trainium2 Bass kernel for a recurrent adaptive-LIF SNN.

Network (per reference):
    B=1024, T=100, n_in=120, h1=512, h2=256, n_out=35
    per step t:
        cur1 = x_t @ W1.T + s1 @ Wrec.T
        a1' = rho1*a1 + (1-rho1)*s1 ; v1' = alpha1*v1*(1-s1) + (1-alpha1)*cur1
        s1' = (v1' - (1 + beta_a1*a1') > 0)
        cur2 = s1' @ W2.T ; same LIF for layer 2
        vo' = beta_out*vo + (1-beta_out)*(s2' @ W3.T) ; out = mean_t vo

Sharding: data-parallel over batch across 8 cores (BC=128 per core),
weights replicated; the sequential T loop is local per core.

Reformulation (shifted potential; all engines in their cheap regimes):
    d := LAM*(v - 1 - beta_a*a)  lives in PSUM (LAM=32 keeps fp8
    stationaries in normal range).  Then
        s'      = (d + C > 0)                      spike, 0/1
        gf'     = relu(-(d + C))                   reset carry = -LAM*min(v'-1-..,0)
        d(t+1)  = LAM(1-a)*W@in + LAM*A*s' + (-a)*gf' + C-in-bias
    with A = -(alpha+cb), cb = beta_a*(1-rho), C = LAM*(alpha-1).
    The reset identity v'*(1-s') <-> min(d,0) makes both nonlinear state
    updates UNARY functions of d, so no tensor-tensor products are needed.
    The residual adaptation-trace couplings ((alpha-rho)*cb*u and
    alpha*cb*u*s', coefficients 7.5e-4 / 7.1e-3) are below the fp8
    quantization noise of the weights and are dropped; they are exactly
    zero whenever the layer is not spiking.
    Output integrator is collapsed into per-step weights:
        out = sum_t w_t * W3 @ s2(t),  w_t = (1-beta_out^(T-t))/T,
    accumulated on the PE as one DoubleRow matmul per step.

All matmuls are fp8e4 DoubleRow (two 128-contract planes per
instruction).  Per step: PE ~23 matmuls; DVE computes s1 (from PSUM) and
s2 (from the SBUF reset carry); ACT computes both reset carries.
"""

import sys
import numpy as np

sys.path.insert(0, "/opt/trn_rl_repo")

import ml_dtypes

F8 = ml_dtypes.float8_e4m3

# Problem constants (hardcoded per contract)
B, T, N_IN, H1, H2, N_OUT = 1024, 100, 120, 512, 256, 35
N_CORES = 8
BC = B // N_CORES  # 128 batch per core
C1 = H1 // 128     # 4 feature chunks, layer 1
C2 = H2 // 128     # 2 feature chunks, layer 2
KX = N_IN // 2     # 60: x is stored as two fp8 DoubleRow planes
LAM = 32.0         # PSUM potential scale
S3 = 256.0         # output stationary scale

_CACHE = {}


def _build(a1, r1, b1, a2, r2, b2, bo):
    import concourse.bacc as bacc
    import concourse.mybir as mybir
    import concourse.tile as tile
    from concourse.alu_op_type import AluOpType

    fp32 = mybir.dt.float32
    fp8 = mybir.dt.float8e4
    A = AluOpType
    DR = mybir.MatmulPerfMode.DoubleRow
    RELU = mybir.ActivationFunctionType.Relu

    C1b = LAM * (a1 - 1.0)   # -1.6 : bias folded into the unary state ops
    C2b = LAM * (a2 - 1.0)

    nc = bacc.Bacc()

    x_d = nc.declare_dram_parameter("x", [KX, T, 2, BC], fp8, isOutput=False)
    w1_d = nc.declare_dram_parameter("w1s", [KX, 2, C1, 128], fp8, isOutput=False)
    wr_d = nc.declare_dram_parameter("wrecs", [128, 2, 2, C1, 128], fp8, isOutput=False)
    w2_d = nc.declare_dram_parameter("w2s", [128, 2, 2, C2, 128], fp8, isOutput=False)
    wd1_d = nc.declare_dram_parameter("wdiag1", [128, 2, 128], fp8, isOutput=False)
    wd2_d = nc.declare_dram_parameter("wdiag2", [128, 2, 128], fp8, isOutput=False)
    w3_d = nc.declare_dram_parameter("w3s", [128, T, 2, 64], fp8, isOutput=False)
    out_d = nc.declare_dram_parameter("out", [N_OUT, BC], fp32, isOutput=True)

    XCH = 10  # x preload chunks
    TP = T // XCH

    with tile.TileContext(nc) as tc:
        with (
            tc.tile_pool(name="wpool", bufs=1) as wpool,
            tc.tile_pool(name="xpool", bufs=1) as xpool,
            tc.tile_pool(name="st1", bufs=2) as st1,
            tc.tile_pool(name="st2", bufs=2) as st2,
            tc.tile_pool(name="tmp", bufs=1) as tmp,
            tc.tile_pool(name="ps1", bufs=2, space="PSUM") as ps1,
            tc.tile_pool(name="ps2", bufs=2, space="PSUM") as ps2,
            tc.tile_pool(name="pso", bufs=1, space="PSUM") as pso,
        ):
            # ---- resident weights ----
            w1_s = wpool.tile([KX, 2, C1, 128], fp8, tag="w1")
            nc.sync.dma_start(w1_s[:], w1_d[:])
            wr_s = wpool.tile([128, 2, 2, C1, 128], fp8, tag="wr")
            nc.sync.dma_start(wr_s[:], wr_d[:])
            w2_s = wpool.tile([128, 2, 2, C2, 128], fp8, tag="w2")
            nc.sync.dma_start(w2_s[:], w2_d[:])
            wd1_s = wpool.tile([128, 2, 128], fp8, tag="wd1")
            nc.sync.dma_start(wd1_s[:], wd1_d[:])
            wd2_s = wpool.tile([128, 2, 128], fp8, tag="wd2")
            nc.sync.dma_start(wd2_s[:], wd2_d[:])
            w3_s = wpool.tile([128, T, 2, 64], fp8, tag="w3")
            nc.sync.dma_start(w3_s[:], w3_d[:])
            bias1_s = wpool.tile([128, 1], fp32, tag="bias1")
            nc.vector.memset(bias1_s[:], -C1b)
            bias2_s = wpool.tile([128, 1], fp32, tag="bias2")
            nc.vector.memset(bias2_s[:], -C2b)

            # ---- x preload in chunks ----
            x_tiles = []
            for i in range(XCH):
                xt = xpool.tile([KX, TP, 2, BC], fp8, tag=f"x{i}")
                nc.sync.dma_start(xt[:], x_d[:, i * TP : (i + 1) * TP, :, :])
                x_tiles.append(xt)

            # ---- initial states: sg[:, k, 0, :]=spike, sg[:, k, 1, :]=reset carry
            sg1 = st1.tile([128, C1, 2, BC], fp8, tag="sg1")
            sg2 = st2.tile([128, C2, 2, BC], fp8, tag="sg2")
            nc.vector.memset(sg1[:], 0.0)
            nc.vector.memset(sg2[:], 0.0)
            nc.vector.memset(sg1[:, :, 1, :], LAM)
            nc.vector.memset(sg2[:, :, 1, :], LAM)

            out_ps = pso.tile([64, BC], fp32, tag="out")

            # x matmuls for step 0 (input-only, no state deps)
            d1 = ps1.tile([128, C1, BC], fp32, tag="d1")
            for m in range(C1):
                nc.tensor.matmul(d1[:, m, :], w1_s[:, :, m, :],
                                 x_tiles[0][:, 0, :, :],
                                 start=True, stop=False, perf_mode=DR)

            sg2_hist = [sg2]
            for t in range(T):
                # ----- d1(t): recurrent + diag terms (x already issued) -----
                for m in range(C1):
                    o = d1[:, m, :]
                    for kp in range(2):
                        nc.tensor.matmul(
                            o, wr_s[:, kp, :, m, :],
                            sg1[:, 2 * kp : 2 * kp + 2, 0, :],
                            start=False, stop=False, perf_mode=DR,
                        )
                    nc.tensor.matmul(o, wd1_s[:], sg1[:, m, :, :],
                                     start=False, stop=True, perf_mode=DR)

                # ----- layer-1 state (unary in d1) -----
                sg1n = st1.tile([128, C1, 2, BC], fp8, tag="sg1")
                nc.vector.tensor_scalar(
                    sg1n[:, :, 0, :], d1[:], C1b, 0.0, A.add, A.is_gt
                )
                nc.scalar.activation(
                    sg1n[:, :, 1, :], d1[:], RELU, bias=bias1_s[:, 0:1], scale=-1.0
                )
                sg1 = sg1n

                # ----- d2(t) -----
                d2 = ps2.tile([128, C2, BC], fp32, tag="d2")
                for m in range(C2):
                    o = d2[:, m, :]
                    for kp in range(2):
                        nc.tensor.matmul(
                            o, w2_s[:, kp, :, m, :],
                            sg1[:, 2 * kp : 2 * kp + 2, 0, :],
                            start=(kp == 0), stop=False, perf_mode=DR,
                        )
                    nc.tensor.matmul(o, wd2_s[:], sg2[:, m, :, :],
                                     start=False, stop=True, perf_mode=DR)

                # ----- layer-2 state: s2 and reset carry both from PSUM -----
                sg2n = st2.tile([128, C2, 2, BC], fp8, tag="sg2")
                nc.vector.tensor_scalar(
                    sg2n[:, :, 0, :], d2[:], C2b, 0.0, A.add, A.is_gt
                )
                nc.scalar.activation(
                    sg2n[:, :, 1, :], d2[:], RELU, bias=bias2_s[:, 0:1], scale=-1.0
                )
                sg2 = sg2n
                sg2_hist.append(sg2)

                # ----- deferred output accumulation for step t-1; keeps the
                # in-order PE stream from stalling on this step's s2 -----
                if t > 0:
                    nc.tensor.matmul(
                        out_ps[:], w3_s[:, t - 1, :, :], sg2_hist[t - 1][:, :, 0, :],
                        start=(t == 1), stop=False, perf_mode=DR,
                        skip_group_check=True,
                    )

                # ----- x matmuls for step t+1 (no state deps) -----
                if t < T - 1:
                    d1 = ps1.tile([128, C1, BC], fp32, tag="d1")
                    xsl = x_tiles[(t + 1) // TP][:, (t + 1) % TP, :, :]
                    for m in range(C1):
                        nc.tensor.matmul(d1[:, m, :], w1_s[:, :, m, :], xsl,
                                         start=True, stop=False, perf_mode=DR)

            nc.tensor.matmul(
                out_ps[:], w3_s[:, T - 1, :, :], sg2_hist[T - 1][:, :, 0, :],
                start=False, stop=True, perf_mode=DR,
                skip_group_check=True,
            )

            outf = tmp.tile([N_OUT, BC], fp32, tag="outf")
            nc.vector.tensor_scalar(outf[:], out_ps[:N_OUT, :], 1.0 / S3, None, A.mult)
            nc.sync.dma_start(out_d[:], outf[:])

    nc.compile()
    return nc


def _prep_inputs(x, W1, Wrec, W2, W3, a1, r1, b1, a2, r2, b2, bo):
    cb1 = b1 * (1.0 - r1)
    cb2 = b2 * (1.0 - r2)
    A1 = -(a1 + cb1)
    A2 = -(a2 + cb2)

    W1f = np.asarray(W1, np.float32) * (LAM * (1.0 - a1))   # [H1, N_IN]
    Wrf = np.asarray(Wrec, np.float32) * (LAM * (1.0 - a1))  # [H1, H1]
    W2f = np.asarray(W2, np.float32) * (LAM * (1.0 - a2))   # [H2, H1]
    W3f = np.asarray(W3, np.float32)                         # [N_OUT, H2]

    # stationary [K, 2, M][k, i, j] = weight(out j, in plane i row k)
    w1s = np.zeros((KX, 2, C1, 128), np.float32)
    for m in range(C1):
        blk = W1f[m * 128 : (m + 1) * 128]          # [128, N_IN]
        w1s[:, 0, m, :] = blk[:, 0:KX].T
        w1s[:, 1, m, :] = blk[:, KX : 2 * KX].T

    wrs = np.zeros((128, 2, 2, C1, 128), np.float32)
    w2s = np.zeros((128, 2, 2, C2, 128), np.float32)
    for m in range(C1):
        blk = Wrf[m * 128 : (m + 1) * 128]          # [128, H1]
        for kp in range(2):
            for i in range(2):
                k = 2 * kp + i
                wrs[:, kp, i, m, :] = blk[:, k * 128 : (k + 1) * 128].T
    for m in range(C2):
        blk = W2f[m * 128 : (m + 1) * 128]          # [128, H1]
        for kp in range(2):
            for i in range(2):
                k = 2 * kp + i
                w2s[:, kp, i, m, :] = blk[:, k * 128 : (k + 1) * 128].T

    eye = np.eye(128, dtype=np.float32)
    wd1 = np.stack([LAM * A1 * eye, -a1 * eye], axis=1)  # [128, 2, 128]
    wd2 = np.stack([LAM * A2 * eye, -a2 * eye], axis=1)

    wt = (1.0 - bo ** (T - np.arange(T, dtype=np.float64))) / T
    w3s = np.zeros((128, T, 2, 64), np.float32)
    for t in range(T):
        sc = np.float32(S3 * wt[t])
        w3s[:, t, 0, :N_OUT] = (sc * W3f[:, 0:128]).T
        w3s[:, t, 1, :N_OUT] = (sc * W3f[:, 128:256]).T

    shared = dict(
        w1s=w1s.astype(F8), wrecs=wrs.astype(F8), w2s=w2s.astype(F8),
        wdiag1=wd1.astype(F8), wdiag2=wd2.astype(F8), w3s=w3s.astype(F8),
    )
    in_maps = []
    for c in range(N_CORES):
        xc = np.asarray(x[c * BC : (c + 1) * BC], np.float32)  # [BC, T, N_IN]
        xfm = xc.transpose(2, 1, 0)                            # [N_IN, T, BC]
        x8 = np.stack([xfm[0:KX], xfm[KX : 2 * KX]], axis=2)   # [KX, T, 2, BC]
        in_maps.append(dict(x=np.ascontiguousarray(x8).astype(F8), **shared))
    return in_maps


def kernel(
    x, W1, Wrec, W2, W3,
    alpha1, rho1, beta_a1, alpha2, rho2, beta_a2, beta_out,
    _trace=False,
):
    from concourse.bass_utils import run_bass_kernel_spmd

    sc = [float(np.asarray(v).reshape(-1)[0]) for v in
          (alpha1, rho1, beta_a1, alpha2, rho2, beta_a2, beta_out)]
    if "nc" not in _CACHE:
        _CACHE["nc"] = _build(*sc)
    nc = _CACHE["nc"]

    in_maps = _prep_inputs(x, W1, Wrec, W2, W3, *sc)
    res = run_bass_kernel_spmd(nc, in_maps, list(range(N_CORES)), trace=_trace)

    out = np.empty((B, N_OUT), np.float32)
    for c in range(N_CORES):
        out[c * BC : (c + 1) * BC] = np.asarray(res.results[c]["out"]).T
    if _trace:
        return out, res
    return out


# revision 8
# speedup vs baseline: 1.3528x; 1.0219x over previous
"""Trainium2 Bass kernel for a recurrent adaptive-LIF SNN.

Network (per reference):
    B=1024, T=100, n_in=120, h1=512, h2=256, n_out=35
    per step t:
        cur1 = x_t @ W1.T + s1 @ Wrec.T
        a1' = rho1*a1 + (1-rho1)*s1 ; v1' = alpha1*v1*(1-s1) + (1-alpha1)*cur1
        s1' = (v1' - (1 + beta_a1*a1') > 0)
        cur2 = s1' @ W2.T ; same LIF for layer 2
        vo' = beta_out*vo + (1-beta_out)*(s2' @ W3.T) ; out = mean_t vo

Sharding: data-parallel over batch across 8 cores (BC=128 per core),
weights replicated; the sequential T loop is local per core.

Reformulation (shifted potential; all engines in their cheap regimes):
    d := LAM*(v - 1 - beta_a*a)  lives in PSUM (LAM=32 keeps fp8
    stationaries in normal range).  Then
        s'      = (d + C > 0)                      spike, 0/1
        gf'     = relu(-(d + C))                   reset carry = -LAM*min(v'-1-..,0)
        d(t+1)  = LAM(1-a)*W@in + LAM*A*s' + (-a)*gf' + C-in-bias
    with A = -(alpha+cb), cb = beta_a*(1-rho), C = LAM*(alpha-1).
    The reset identity v'*(1-s') <-> min(d,0) makes both nonlinear state
    updates UNARY functions of d, so no tensor-tensor products are needed.
    The residual adaptation-trace couplings ((alpha-rho)*cb*u and
    alpha*cb*u*s', coefficients 7.5e-4 / 7.1e-3) are below the fp8
    quantization noise of the weights and are dropped; they are exactly
    zero whenever the layer is not spiking.
    Output integrator is collapsed into per-step weights:
        out = sum_t w_t * W3 @ s2(t),  w_t = (1-beta_out^(T-t))/T,
    accumulated on the PE as one DoubleRow matmul per step.

All matmuls are fp8e4 DoubleRow (two 128-contract planes per
instruction).  Per step: PE ~23 matmuls; DVE computes s1 (from PSUM) and
s2 (from the SBUF reset carry); ACT computes both reset carries.
"""

import sys
import numpy as np

sys.path.insert(0, "/opt/trn_rl_repo")

import ml_dtypes

F8 = ml_dtypes.float8_e4m3

# Problem constants (hardcoded per contract)
B, T, N_IN, H1, H2, N_OUT = 1024, 100, 120, 512, 256, 35
N_CORES = 8
BC = B // N_CORES  # 128 batch per core
C1 = H1 // 128     # 4 feature chunks, layer 1
C2 = H2 // 128     # 2 feature chunks, layer 2
KX = N_IN // 2     # 60: x is stored as two fp8 DoubleRow planes
LAM = 32.0         # PSUM potential scale
S3 = 256.0         # output stationary scale

_CACHE = {}


def _build(a1, r1, b1, a2, r2, b2, bo):
    import concourse.bacc as bacc
    import concourse.mybir as mybir
    import concourse.tile as tile
    from concourse.alu_op_type import AluOpType

    fp32 = mybir.dt.float32
    fp8 = mybir.dt.float8e4
    A = AluOpType
    DR = mybir.MatmulPerfMode.DoubleRow
    RELU = mybir.ActivationFunctionType.Relu

    C1b = LAM * (a1 - 1.0)   # -1.6 : bias folded into the unary state ops
    C2b = LAM * (a2 - 1.0)

    nc = bacc.Bacc()

    x_d = nc.declare_dram_parameter("x", [KX, T, 2, BC], fp8, isOutput=False)
    w1_d = nc.declare_dram_parameter("w1s", [KX, 2, C1, 128], fp8, isOutput=False)
    wr_d = nc.declare_dram_parameter("wrecs", [128, 2, 2, C1, 128], fp8, isOutput=False)
    w2_d = nc.declare_dram_parameter("w2s", [128, 2, 2, C2, 128], fp8, isOutput=False)
    wdg1_d = nc.declare_dram_parameter("wdiag_g1", [128, 2, 2, 128], fp8, isOutput=False)
    wds2_d = nc.declare_dram_parameter("wdiag_s2", [128, 2, 2, 128], fp8, isOutput=False)
    wdg2_d = nc.declare_dram_parameter("wdiag_g2", [128, 2, 2, 128], fp8, isOutput=False)
    w3_d = nc.declare_dram_parameter("w3s", [128, T, 2, 64], fp8, isOutput=False)
    out_d = nc.declare_dram_parameter("out", [N_OUT, BC], fp32, isOutput=True)

    XCH = 10  # x preload chunks
    TP = T // XCH

    with tile.TileContext(nc) as tc:
        with (
            tc.tile_pool(name="wpool", bufs=1) as wpool,
            tc.tile_pool(name="xpool", bufs=1) as xpool,
            tc.tile_pool(name="st1", bufs=2) as st1,
            tc.tile_pool(name="st2", bufs=2) as st2,
            tc.tile_pool(name="tmp", bufs=1) as tmp,
            tc.tile_pool(name="ps1", bufs=2, space="PSUM") as ps1,
            tc.tile_pool(name="ps2", bufs=2, space="PSUM") as ps2,
            tc.tile_pool(name="pso", bufs=1, space="PSUM") as pso,
        ):
            # ---- resident weights ----
            w1_s = wpool.tile([KX, 2, C1, 128], fp8, tag="w1")
            nc.sync.dma_start(w1_s[:], w1_d[:])
            wr_s = wpool.tile([128, 2, 2, C1, 128], fp8, tag="wr")
            nc.sync.dma_start(wr_s[:], wr_d[:])
            w2_s = wpool.tile([128, 2, 2, C2, 128], fp8, tag="w2")
            nc.sync.dma_start(w2_s[:], w2_d[:])
            wdg1_s, wds2_s, wdg2_s = [], [], []
            for p in range(2):
                a = wpool.tile([128, 2, 128], fp8, tag=f"wdg1_{p}")
                nc.sync.dma_start(a[:], wdg1_d[:, p])
                wdg1_s.append(a)
                b = wpool.tile([128, 2, 128], fp8, tag=f"wds2_{p}")
                nc.sync.dma_start(b[:], wds2_d[:, p])
                wds2_s.append(b)
                c = wpool.tile([128, 2, 128], fp8, tag=f"wdg2_{p}")
                nc.sync.dma_start(c[:], wdg2_d[:, p])
                wdg2_s.append(c)
            w3_s = wpool.tile([128, T, 2, 64], fp8, tag="w3")
            nc.sync.dma_start(w3_s[:], w3_d[:])
            bias1_s = wpool.tile([128, 1], fp32, tag="bias1")
            nc.vector.memset(bias1_s[:], -C1b)
            bias2_s = wpool.tile([128, 1], fp32, tag="bias2")
            nc.vector.memset(bias2_s[:], -C2b)

            # ---- x preload in chunks ----
            x_tiles = []
            for i in range(XCH):
                xt = xpool.tile([KX, TP, 2, BC], fp8, tag=f"x{i}")
                nc.sync.dma_start(xt[:], x_d[:, i * TP : (i + 1) * TP, :, :])
                x_tiles.append(xt)

            # ---- states: separate single-writer tensors, 2 slots each ----
            # slot t%2 holds step-t state; diag matmuls use DR pairs over the
            # two slots with a zero stationary on the stale slot.
            s1w = st1.tile([128, C1, 2, BC], fp8, tag="s1w")
            g1w = st1.tile([128, C1, 2, BC], fp8, tag="g1w")
            s2w = st2.tile([128, C2, 2, BC], fp8, tag="s2w")
            g2w = st2.tile([128, C2, 2, BC], fp8, tag="g2w")
            nc.vector.memset(s1w[:], 0.0)
            nc.vector.memset(s2w[:], 0.0)
            nc.vector.memset(g1w[:], LAM)
            nc.vector.memset(g2w[:], LAM)

            out_ps = pso.tile([64, BC], fp32, tag="out")

            # x matmuls for step 0 (input-only, no state deps)
            d1 = ps1.tile([128, C1, BC], fp32, tag="d1")
            for m in range(C1):
                nc.tensor.matmul(d1[:, m, :], w1_s[:, :, m, :],
                                 x_tiles[0][:, 0, :, :],
                                 start=True, stop=False, perf_mode=DR)

            for t in range(T):
                pv = (t - 1) % 2   # slot holding step t-1 state
                cu = t % 2         # slot this step writes

                # ----- d1(t): recurrent + diag terms (x already issued) -----
                for m in range(C1):
                    o = d1[:, m, :]
                    for kp in range(2):
                        nc.tensor.matmul(
                            o, wr_s[:, kp, :, m, :],
                            s1w[:, 2 * kp : 2 * kp + 2, pv, :],
                            start=False, stop=False, perf_mode=DR,
                        )
                    nc.tensor.matmul(o, wdg1_s[pv][:], g1w[:, m, :, :],
                                     start=False, stop=True, perf_mode=DR)

                # ----- layer-1 state (unary in d1; independent writers) -----
                nc.vector.tensor_scalar(
                    s1w[:, :, cu, :], d1[:], C1b, 0.0, A.add, A.is_gt
                )
                nc.scalar.activation(
                    g1w[:, :, cu, :], d1[:], RELU, bias=bias1_s[:, 0:1], scale=-1.0
                )

                # ----- d2(t) -----
                d2 = ps2.tile([128, C2, BC], fp32, tag="d2")
                for m in range(C2):
                    o = d2[:, m, :]
                    for kp in range(2):
                        nc.tensor.matmul(
                            o, w2_s[:, kp, :, m, :],
                            s1w[:, 2 * kp : 2 * kp + 2, cu, :],
                            start=(kp == 0), stop=False, perf_mode=DR,
                        )
                    nc.tensor.matmul(o, wds2_s[pv][:], s2w[:, m, :, :],
                                     start=False, stop=False, perf_mode=DR)
                    nc.tensor.matmul(o, wdg2_s[pv][:], g2w[:, m, :, :],
                                     start=False, stop=True, perf_mode=DR)

                # ----- layer-2 state -----
                nc.vector.tensor_scalar(
                    s2w[:, :, cu, :], d2[:], C2b, 0.0, A.add, A.is_gt
                )
                nc.scalar.activation(
                    g2w[:, :, cu, :], d2[:], RELU, bias=bias2_s[:, 0:1], scale=-1.0
                )

                # ----- deferred output accumulation for step t-1; keeps the
                # in-order PE stream from stalling on this step's s2 -----
                if t > 0:
                    nc.tensor.matmul(
                        out_ps[:], w3_s[:, t - 1, :, :], s2w[:, 0:2, pv, :],
                        start=(t == 1), stop=False, perf_mode=DR,
                        skip_group_check=True,
                    )

                # ----- x matmuls for step t+1 (no state deps) -----
                if t < T - 1:
                    d1 = ps1.tile([128, C1, BC], fp32, tag="d1")
                    xsl = x_tiles[(t + 1) // TP][:, (t + 1) % TP, :, :]
                    for m in range(C1):
                        nc.tensor.matmul(d1[:, m, :], w1_s[:, :, m, :], xsl,
                                         start=True, stop=False, perf_mode=DR)

            nc.tensor.matmul(
                out_ps[:], w3_s[:, T - 1, :, :], s2w[:, 0:2, (T - 1) % 2, :],
                start=False, stop=True, perf_mode=DR,
                skip_group_check=True,
            )

            outf = tmp.tile([N_OUT, BC], fp32, tag="outf")
            nc.vector.tensor_scalar(outf[:], out_ps[:N_OUT, :], 1.0 / S3, None, A.mult)
            nc.sync.dma_start(out_d[:], outf[:])

    nc.compile()
    return nc


def _prep_inputs(x, W1, Wrec, W2, W3, a1, r1, b1, a2, r2, b2, bo):
    cb1 = b1 * (1.0 - r1)
    cb2 = b2 * (1.0 - r2)
    A1 = -(a1 + cb1)
    A2 = -(a2 + cb2)

    W1f = np.asarray(W1, np.float32) * (LAM * (1.0 - a1))   # [H1, N_IN]
    Wrf = np.asarray(Wrec, np.float32) * (LAM * (1.0 - a1)) \
        + (LAM * A1) * np.eye(H1, dtype=np.float32)              # [H1, H1]
    W2f = np.asarray(W2, np.float32) * (LAM * (1.0 - a2))   # [H2, H1]
    W3f = np.asarray(W3, np.float32)                         # [N_OUT, H2]

    # stationary [K, 2, M][k, i, j] = weight(out j, in plane i row k)
    w1s = np.zeros((KX, 2, C1, 128), np.float32)
    for m in range(C1):
        blk = W1f[m * 128 : (m + 1) * 128]          # [128, N_IN]
        w1s[:, 0, m, :] = blk[:, 0:KX].T
        w1s[:, 1, m, :] = blk[:, KX : 2 * KX].T

    wrs = np.zeros((128, 2, 2, C1, 128), np.float32)
    w2s = np.zeros((128, 2, 2, C2, 128), np.float32)
    for m in range(C1):
        blk = Wrf[m * 128 : (m + 1) * 128]          # [128, H1]
        for kp in range(2):
            for i in range(2):
                k = 2 * kp + i
                wrs[:, kp, i, m, :] = blk[:, k * 128 : (k + 1) * 128].T
    for m in range(C2):
        blk = W2f[m * 128 : (m + 1) * 128]          # [128, H1]
        for kp in range(2):
            for i in range(2):
                k = 2 * kp + i
                w2s[:, kp, i, m, :] = blk[:, k * 128 : (k + 1) * 128].T

    eye = np.eye(128, dtype=np.float32)
    zz = np.zeros_like(eye)
    # [128, parity, slotplane(2), 128]: parity p selects slot p as live
    def slotdiag(c):
        return np.stack([np.stack([c * eye, zz], axis=1),
                         np.stack([zz, c * eye], axis=1)], axis=1)
    wdg1 = slotdiag(-a1)
    wds2 = slotdiag(LAM * A2)
    wdg2 = slotdiag(-a2)

    wt = (1.0 - bo ** (T - np.arange(T, dtype=np.float64))) / T
    w3s = np.zeros((128, T, 2, 64), np.float32)
    for t in range(T):
        sc = np.float32(S3 * wt[t])
        w3s[:, t, 0, :N_OUT] = (sc * W3f[:, 0:128]).T
        w3s[:, t, 1, :N_OUT] = (sc * W3f[:, 128:256]).T

    shared = dict(
        w1s=w1s.astype(F8), wrecs=wrs.astype(F8), w2s=w2s.astype(F8),
        wdiag_g1=wdg1.astype(F8), wdiag_s2=wds2.astype(F8),
        wdiag_g2=wdg2.astype(F8), w3s=w3s.astype(F8),
    )
    in_maps = []
    for c in range(N_CORES):
        xc = np.asarray(x[c * BC : (c + 1) * BC], np.float32)  # [BC, T, N_IN]
        xfm = xc.transpose(2, 1, 0)                            # [N_IN, T, BC]
        x8 = np.stack([xfm[0:KX], xfm[KX : 2 * KX]], axis=2)   # [KX, T, 2, BC]
        in_maps.append(dict(x=np.ascontiguousarray(x8).astype(F8), **shared))
    return in_maps


def kernel(
    x, W1, Wrec, W2, W3,
    alpha1, rho1, beta_a1, alpha2, rho2, beta_a2, beta_out,
    _trace=False,
):
    from concourse.bass_utils import run_bass_kernel_spmd

    sc = [float(np.asarray(v).reshape(-1)[0]) for v in
          (alpha1, rho1, beta_a1, alpha2, rho2, beta_a2, beta_out)]
    if "nc" not in _CACHE:
        _CACHE["nc"] = _build(*sc)
    nc = _CACHE["nc"]

    in_maps = _prep_inputs(x, W1, Wrec, W2, W3, *sc)
    res = run_bass_kernel_spmd(nc, in_maps, list(range(N_CORES)), trace=_trace)

    out = np.empty((B, N_OUT), np.float32)
    for c in range(N_CORES):
        out[c * BC : (c + 1) * BC] = np.asarray(res.results[c]["out"]).T
    if _trace:
        return out, res
    return out


# revision 9
# speedup vs baseline: 1.4862x; 1.0986x over previous
"""Trainium2 Bass kernel for a recurrent adaptive-LIF SNN.

Network (per reference):
    B=1024, T=100, n_in=120, h1=512, h2=256, n_out=35
    per step t:
        cur1 = x_t @ W1.T + s1 @ Wrec.T
        a1' = rho1*a1 + (1-rho1)*s1 ; v1' = alpha1*v1*(1-s1) + (1-alpha1)*cur1
        s1' = (v1' - (1 + beta_a1*a1') > 0)
        cur2 = s1' @ W2.T ; same LIF for layer 2
        vo' = beta_out*vo + (1-beta_out)*(s2' @ W3.T) ; out = mean_t vo

Sharding: data-parallel over batch across 8 cores (BC=128 per core),
weights replicated; the sequential T loop is local per core.

Reformulation (shifted potential; all engines in their cheap regimes):
    d := LAM*(v - 1 - beta_a*a)  lives in PSUM (LAM=32 keeps fp8
    stationaries in normal range).  Then
        s'      = (d + C > 0)                      spike, 0/1
        gf'     = relu(-(d + C))                   reset carry = -LAM*min(v'-1-..,0)
        d(t+1)  = LAM(1-a)*W@in + LAM*A*s' + (-a)*gf' + C-in-bias
    with A = -(alpha+cb), cb = beta_a*(1-rho), C = LAM*(alpha-1).
    The reset identity v'*(1-s') <-> min(d,0) makes both nonlinear state
    updates UNARY functions of d, so no tensor-tensor products are needed.
    The residual adaptation-trace couplings ((alpha-rho)*cb*u and
    alpha*cb*u*s', coefficients 7.5e-4 / 7.1e-3) are below the fp8
    quantization noise of the weights and are dropped; they are exactly
    zero whenever the layer is not spiking.
    Output integrator is collapsed into per-step weights:
        out = sum_t w_t * W3 @ s2(t),  w_t = (1-beta_out^(T-t))/T,
    accumulated on the PE as one DoubleRow matmul per step.

All matmuls are fp8e4 DoubleRow (two 128-contract planes per
instruction).  Per step: PE ~23 matmuls; DVE computes s1 (from PSUM) and
s2 (from the SBUF reset carry); ACT computes both reset carries.
"""

import sys
import numpy as np

sys.path.insert(0, "/opt/trn_rl_repo")

import ml_dtypes

F8 = ml_dtypes.float8_e4m3

# Problem constants (hardcoded per contract)
B, T, N_IN, H1, H2, N_OUT = 1024, 100, 120, 512, 256, 35
N_CORES = 8
BC = B // N_CORES  # 128 batch per core
C1 = H1 // 128     # 4 feature chunks, layer 1
C2 = H2 // 128     # 2 feature chunks, layer 2
KX = N_IN // 2     # 60: x is stored as two fp8 DoubleRow planes
LAM = 32.0         # PSUM potential scale
S3 = 256.0         # output stationary scale

_CACHE = {}


def _build(a1, r1, b1, a2, r2, b2, bo):
    import concourse.bacc as bacc
    import concourse.mybir as mybir
    import concourse.tile as tile
    from concourse.alu_op_type import AluOpType

    fp32 = mybir.dt.float32
    fp8 = mybir.dt.float8e4
    A = AluOpType
    DR = mybir.MatmulPerfMode.DoubleRow
    RELU = mybir.ActivationFunctionType.Relu

    C1b = LAM * (a1 - 1.0)   # -1.6 : bias folded into the unary state ops
    C2b = LAM * (a2 - 1.0)

    nc = bacc.Bacc()

    x_d = nc.declare_dram_parameter("x", [KX, T, 2, BC], fp8, isOutput=False)
    w1_d = nc.declare_dram_parameter("w1s", [KX, 2, C1, 128], fp8, isOutput=False)
    wr_d = nc.declare_dram_parameter("wrecs", [128, 2, 2, C1, 128], fp8, isOutput=False)
    w2_d = nc.declare_dram_parameter("w2s", [128, 2, 2, C2, 128], fp8, isOutput=False)
    wdg1_d = nc.declare_dram_parameter("wdiag_g1", [128, 128], fp8, isOutput=False)
    wds2_d = nc.declare_dram_parameter("wdiag_s2", [128, 128], fp8, isOutput=False)
    wdg2_d = nc.declare_dram_parameter("wdiag_g2", [128, 128], fp8, isOutput=False)
    w3_d = nc.declare_dram_parameter("w3s", [128, T, 2, 64], fp8, isOutput=False)
    out_d = nc.declare_dram_parameter("out", [N_OUT, BC], fp32, isOutput=True)

    XCH = 10  # x preload chunks
    TP = T // XCH

    with tile.TileContext(nc) as tc:
        with (
            tc.tile_pool(name="wpool", bufs=1) as wpool,
            tc.tile_pool(name="xpool", bufs=1) as xpool,
            tc.tile_pool(name="st1", bufs=2) as st1,
            tc.tile_pool(name="st2", bufs=2) as st2,
            tc.tile_pool(name="tmp", bufs=1) as tmp,
            tc.tile_pool(name="ps1", bufs=2, space="PSUM") as ps1,
            tc.tile_pool(name="ps2", bufs=2, space="PSUM") as ps2,
            tc.tile_pool(name="pso", bufs=1, space="PSUM") as pso,
        ):
            # ---- resident weights ----
            w1_s = wpool.tile([KX, 2, C1, 128], fp8, tag="w1")
            nc.sync.dma_start(w1_s[:], w1_d[:])
            wr_s = wpool.tile([128, 2, 2, C1, 128], fp8, tag="wr")
            nc.sync.dma_start(wr_s[:], wr_d[:])
            w2_s = wpool.tile([128, 2, 2, C2, 128], fp8, tag="w2")
            nc.sync.dma_start(w2_s[:], w2_d[:])
            wdg1_s = wpool.tile([128, 128], fp8, tag="wdg1")
            nc.sync.dma_start(wdg1_s[:], wdg1_d[:])
            wds2_s = wpool.tile([128, 128], fp8, tag="wds2")
            nc.sync.dma_start(wds2_s[:], wds2_d[:])
            wdg2_s = wpool.tile([128, 128], fp8, tag="wdg2")
            nc.sync.dma_start(wdg2_s[:], wdg2_d[:])
            w3_s = wpool.tile([128, T, 2, 64], fp8, tag="w3")
            nc.sync.dma_start(w3_s[:], w3_d[:])
            bias1_s = wpool.tile([128, 1], fp32, tag="bias1")
            nc.vector.memset(bias1_s[:], -C1b)
            bias2_s = wpool.tile([128, 1], fp32, tag="bias2")
            nc.vector.memset(bias2_s[:], -C2b)

            # ---- x preload in chunks ----
            x_tiles = []
            for i in range(XCH):
                xt = xpool.tile([KX, TP, 2, BC], fp8, tag=f"x{i}")
                nc.sync.dma_start(xt[:], x_d[:, i * TP : (i + 1) * TP, :, :])
                x_tiles.append(xt)

            # ---- states: single-writer pool-rotated tiles ----
            s1 = st1.tile([128, C1, BC], fp8, tag="s1")
            g1 = st1.tile([128, C1, BC], fp8, tag="g1")
            s2 = st2.tile([128, C2, BC], fp8, tag="s2")
            g2 = st2.tile([128, C2, BC], fp8, tag="g2")
            nc.vector.memset(s1[:], 0.0)
            nc.vector.memset(s2[:], 0.0)
            nc.vector.memset(g1[:], LAM)
            nc.vector.memset(g2[:], LAM)

            out_ps = pso.tile([64, BC], fp32, tag="out")

            # x matmuls for step 0 (input-only, no state deps)
            d1 = ps1.tile([128, C1, BC], fp32, tag="d1")
            for m in range(C1):
                nc.tensor.matmul(d1[:, m, :], w1_s[:, :, m, :],
                                 x_tiles[0][:, 0, :, :],
                                 start=True, stop=False, perf_mode=DR)

            for t in range(T):
                # ----- d1(t): recurrent + diag terms (x already issued) -----
                for m in range(C1):
                    o = d1[:, m, :]
                    for kp in range(2):
                        nc.tensor.matmul(
                            o, wr_s[:, kp, :, m, :],
                            s1[:, 2 * kp : 2 * kp + 2, :],
                            start=False, stop=False, perf_mode=DR,
                        )
                # reset-carry diagonal: one plain matmul across all chunks
                nc.tensor.matmul(d1[:, :, :], wdg1_s[:], g1[:, :, :],
                                 start=False, stop=True, skip_group_check=True)

                # ----- layer-1 state (unary in d1; independent writers) -----
                s1 = st1.tile([128, C1, BC], fp8, tag="s1")
                g1 = st1.tile([128, C1, BC], fp8, tag="g1")
                nc.vector.tensor_scalar(
                    s1[:], d1[:], C1b, 0.0, A.add, A.is_gt
                )
                nc.scalar.activation(
                    g1[:], d1[:], RELU, bias=bias1_s[:, 0:1], scale=-1.0
                )

                # ----- d2(t) -----
                d2 = ps2.tile([128, C2, BC], fp32, tag="d2")
                for m in range(C2):
                    o = d2[:, m, :]
                    for kp in range(2):
                        nc.tensor.matmul(
                            o, w2_s[:, kp, :, m, :],
                            s1[:, 2 * kp : 2 * kp + 2, :],
                            start=(kp == 0), stop=False, perf_mode=DR,
                        )
                nc.tensor.matmul(d2[:, :, :], wds2_s[:], s2[:, :, :],
                                 start=False, stop=False, skip_group_check=True)
                nc.tensor.matmul(d2[:, :, :], wdg2_s[:], g2[:, :, :],
                                 start=False, stop=True, skip_group_check=True)

                # ----- layer-2 state -----
                s2 = st2.tile([128, C2, BC], fp8, tag="s2")
                g2 = st2.tile([128, C2, BC], fp8, tag="g2")
                nc.vector.tensor_scalar(
                    s2[:], d2[:], C2b, 0.0, A.add, A.is_gt
                )
                nc.scalar.activation(
                    g2[:], d2[:], RELU, bias=bias2_s[:, 0:1], scale=-1.0
                )

                # ----- deferred output accumulation for step t-1 -----
                if t > 0:
                    nc.tensor.matmul(
                        out_ps[:], w3_s[:, t - 1, :, :], s2_pv[:, 0:2, :],
                        start=(t == 1), stop=False, perf_mode=DR,
                        skip_group_check=True,
                    )
                s2_pv = s2

                # ----- x matmuls for step t+1 (no state deps) -----
                if t < T - 1:
                    d1 = ps1.tile([128, C1, BC], fp32, tag="d1")
                    xsl = x_tiles[(t + 1) // TP][:, (t + 1) % TP, :, :]
                    for m in range(C1):
                        nc.tensor.matmul(d1[:, m, :], w1_s[:, :, m, :], xsl,
                                         start=True, stop=False, perf_mode=DR)

            nc.tensor.matmul(
                out_ps[:], w3_s[:, T - 1, :, :], s2_pv[:, 0:2, :],
                start=False, stop=True, perf_mode=DR,
                skip_group_check=True,
            )

            outf = tmp.tile([N_OUT, BC], fp32, tag="outf")
            nc.vector.tensor_scalar(outf[:], out_ps[:N_OUT, :], 1.0 / S3, None, A.mult)
            nc.sync.dma_start(out_d[:], outf[:])

    nc.compile()
    return nc


def _prep_inputs(x, W1, Wrec, W2, W3, a1, r1, b1, a2, r2, b2, bo):
    cb1 = b1 * (1.0 - r1)
    cb2 = b2 * (1.0 - r2)
    A1 = -(a1 + cb1)
    A2 = -(a2 + cb2)

    W1f = np.asarray(W1, np.float32) * (LAM * (1.0 - a1))   # [H1, N_IN]
    Wrf = np.asarray(Wrec, np.float32) * (LAM * (1.0 - a1)) \
        + (LAM * A1) * np.eye(H1, dtype=np.float32)              # [H1, H1]
    W2f = np.asarray(W2, np.float32) * (LAM * (1.0 - a2))   # [H2, H1]
    W3f = np.asarray(W3, np.float32)                         # [N_OUT, H2]

    # stationary [K, 2, M][k, i, j] = weight(out j, in plane i row k)
    w1s = np.zeros((KX, 2, C1, 128), np.float32)
    for m in range(C1):
        blk = W1f[m * 128 : (m + 1) * 128]          # [128, N_IN]
        w1s[:, 0, m, :] = blk[:, 0:KX].T
        w1s[:, 1, m, :] = blk[:, KX : 2 * KX].T

    wrs = np.zeros((128, 2, 2, C1, 128), np.float32)
    w2s = np.zeros((128, 2, 2, C2, 128), np.float32)
    for m in range(C1):
        blk = Wrf[m * 128 : (m + 1) * 128]          # [128, H1]
        for kp in range(2):
            for i in range(2):
                k = 2 * kp + i
                wrs[:, kp, i, m, :] = blk[:, k * 128 : (k + 1) * 128].T
    for m in range(C2):
        blk = W2f[m * 128 : (m + 1) * 128]          # [128, H1]
        for kp in range(2):
            for i in range(2):
                k = 2 * kp + i
                w2s[:, kp, i, m, :] = blk[:, k * 128 : (k + 1) * 128].T

    eye = np.eye(128, dtype=np.float32)
    wdg1 = -a1 * eye
    wds2 = LAM * A2 * eye
    wdg2 = -a2 * eye

    wt = (1.0 - bo ** (T - np.arange(T, dtype=np.float64))) / T
    w3s = np.zeros((128, T, 2, 64), np.float32)
    for t in range(T):
        sc = np.float32(S3 * wt[t])
        w3s[:, t, 0, :N_OUT] = (sc * W3f[:, 0:128]).T
        w3s[:, t, 1, :N_OUT] = (sc * W3f[:, 128:256]).T

    shared = dict(
        w1s=w1s.astype(F8), wrecs=wrs.astype(F8), w2s=w2s.astype(F8),
        wdiag_g1=wdg1.astype(F8), wdiag_s2=wds2.astype(F8),
        wdiag_g2=wdg2.astype(F8), w3s=w3s.astype(F8),
    )
    in_maps = []
    for c in range(N_CORES):
        xc = np.asarray(x[c * BC : (c + 1) * BC], np.float32)  # [BC, T, N_IN]
        xfm = xc.transpose(2, 1, 0)                            # [N_IN, T, BC]
        x8 = np.stack([xfm[0:KX], xfm[KX : 2 * KX]], axis=2)   # [KX, T, 2, BC]
        in_maps.append(dict(x=np.ascontiguousarray(x8).astype(F8), **shared))
    return in_maps


def kernel(
    x, W1, Wrec, W2, W3,
    alpha1, rho1, beta_a1, alpha2, rho2, beta_a2, beta_out,
    _trace=False,
):
    from concourse.bass_utils import run_bass_kernel_spmd

    sc = [float(np.asarray(v).reshape(-1)[0]) for v in
          (alpha1, rho1, beta_a1, alpha2, rho2, beta_a2, beta_out)]
    if "nc" not in _CACHE:
        _CACHE["nc"] = _build(*sc)
    nc = _CACHE["nc"]

    in_maps = _prep_inputs(x, W1, Wrec, W2, W3, *sc)
    res = run_bass_kernel_spmd(nc, in_maps, list(range(N_CORES)), trace=_trace)

    out = np.empty((B, N_OUT), np.float32)
    for c in range(N_CORES):
        out[c * BC : (c + 1) * BC] = np.asarray(res.results[c]["out"]).T
    if _trace:
        return out, res
    return out
